# revision 15
# baseline (speedup 1.0000x reference)
"""Bidirectional autoregressive Mamba (2 layers) on 8 Trainium2 NeuronCores.

Sharding: tensor-parallel over d_inner (2048 -> 256/core) and MLP hidden
(2048 -> 256/core); the sequence stays local (selective-scan recurrence).
Cross-core AllReduce after x_proj / out_proj / fc2. Activations are kept
F-major (features on partitions, tokens on the free axis) end to end.

The selective scan is a loop over the 64 state dims: per state n the
per-channel recurrence h = exp(A[n]*dt)*h + (dt*u)*B_t[n] is one DVE
tensor_tensor_scan over a (128 channels x T) tile; the B_t/C_t rows are
partition-broadcast via K=1 matmuls on the tensor engine into PSUM.
Layer 1 (the flipped layer) runs the scan right-to-left via negative-stride
APs and an anticausal conv, so nothing is physically flipped on device; the
host flips the second output at the end.
"""

import numpy as np

import concourse.bass as bass
import concourse.mybir as mybir
from concourse import tile
from concourse.bass_utils import run_bass_kernel_spmd

F32 = mybir.dt.float32
F32R = mybir.dt.float32r
OP = mybir.AluOpType
AT = mybir.ActivationFunctionType

D_MODEL = 1024
D_STATE = 64
D_CONV = 4
D_INNER = 2048
DT_RANK = 64
HIDDEN = 2048
RMS_EPS = 1e-5
LN_EPS = 1e-5
NCORES = 8
DLOC = D_INNER // NCORES   # 256 channels/core -> 2 partition tiles
HLOC = HIDDEN // NCORES

MM_MODE = "f32"   # "f32r" full-rate fp32 matmul, or "f32" quarter-rate


def _mm(ap):
    return ap.bitcast(F32R) if MM_MODE == "f32r" else ap


def _chunks(T):
    return [(s, min(s + 512, T)) for s in range(0, T, 512)]


def build_kernel(nc, T, an_values, debug=False):
    dp = nc.declare_dram_parameter
    NP = D_MODEL // 128          # 8 feature tiles
    CH = _chunks(T)
    NCH = len(CH)
    TPAD = 512 * NCH

    x_ext = dp("xT", [D_MODEL, T], F32, isOutput=False)
    onesr_ext = dp("ones_r", [128, 128], F32, isOutput=False)
    onesc_ext = dp("ones_c", [128, 1], F32, isOutput=False)
    lnw_ext = dp("lnw", [D_MODEL, 1], F32, isOutput=False)
    lnb_ext = dp("lnb", [D_MODEL, 1], F32, isOutput=False)
    wext = {}
    for l in range(2):
        for nm, shp in [
            ("w_xm", [D_MODEL, DLOC]), ("w_z", [D_MODEL, DLOC]),
            ("convw", [DLOC, D_CONV]), ("convb", [DLOC, 1]),
            ("xproj", [DLOC, DT_RANK + 2 * D_STATE]),
            ("dtw", [DT_RANK, DLOC]), ("dtb", [DLOC, 1]), ("dvec", [DLOC, 1]),
            ("wout", [DLOC, D_MODEL]),
            ("fc1y", [D_MODEL, HLOC]), ("fc1g", [D_MODEL, HLOC]),
            ("fc1by", [HLOC, 1]), ("fc1bg", [HLOC, 1]),
            ("fc2", [HLOC, D_MODEL]), ("fc2b", [D_MODEL, 1]),
        ]:
            wext[(l, nm)] = dp(f"L{l}_{nm}", shp, F32, isOutput=False)
    y0_ext = dp("y0T", [D_MODEL, T], F32, isOutput=True)
    y1_ext = dp("y1T", [D_MODEL, T], F32, isOutput=True)

    dbg_outs = {}

    def _mkdbg(name, shape):
        dbg_outs[name] = dp("dbg_" + name, list(shape), F32, isOutput=True)

    with tile.TileContext(nc) as tc:
        with (
            tc.tile_pool(name="res", bufs=1) as res_pool,
            tc.tile_pool(name="act", bufs=1) as act_pool,
            tc.tile_pool(name="wts", bufs=2) as wts_pool,
            tc.tile_pool(name="scr", bufs=1) as scr_pool,
            tc.tile_pool(name="sc2", bufs=2) as sc2_pool,
            tc.tile_pool(name="ps", bufs=2, space="PSUM") as ps_pool,
            tc.tile_pool(name="psb", bufs=1, space="PSUM") as psb_pool,
            tc.tile_pool(name="dr", bufs=1, space="DRAM") as dram_pool,
        ):
            R = [res_pool.tile([128, T], F32, name=f"res{j}", tag=f"res{j}")
                 for j in range(NP)]
            ones_r = res_pool.tile([128, 128], F32, name="ones_r", tag="ones_r")
            ones_c = res_pool.tile([128, 1], F32, name="ones_c", tag="ones_c")
            nc.sync.dma_start(ones_r[:], onesr_ext[:])
            nc.sync.dma_start(ones_c[:], onesc_ext[:])
            lnw = [res_pool.tile([128, 1], F32, name=f"lnw{j}", tag=f"lnw{j}") for j in range(NP)]
            lnb = [res_pool.tile([128, 1], F32, name=f"lnb{j}", tag=f"lnb{j}") for j in range(NP)]
            for j in range(NP):
                nc.sync.dma_start(R[j][:], x_ext[128 * j:128 * (j + 1), :])
                nc.sync.dma_start(lnw[j][:], lnw_ext[128 * j:128 * (j + 1), :])
                nc.sync.dma_start(lnb[j][:], lnb_ext[128 * j:128 * (j + 1), :])

            eps_rms = res_pool.tile([1, 1], F32, name="eps_rms", tag="eps_rms")
            eps_ln = res_pool.tile([1, 1], F32, name="eps_ln", tag="eps_ln")
            nc.gpsimd.memset(eps_rms[:], RMS_EPS)
            nc.gpsimd.memset(eps_ln[:], LN_EPS)

            ar_in = dram_pool.tile([D_MODEL, T], F32, name="ar_in", tag="ar_in")
            ar_out = dram_pool.tile([D_MODEL, T], F32, name="ar_out", tag="ar_out")
            bc_in = dram_pool.tile([192, T], F32, name="bc_in", tag="bc_in")
            bc_out = dram_pool.tile([192, T], F32, name="bc_out", tag="bc_out")

            def bcast_row(row_ap, tag):
                """(1,T) row (base partition 0) -> (128,TPAD) PSUM tile."""
                pb = psb_pool.tile([128, TPAD], F32, name=tag, tag=tag)
                for (s, e) in CH:
                    nc.tensor.matmul(pb[:, s:e], _mm(ones_r[0:1, :]),
                                     _mm(row_ap[:, s:e]), start=True, stop=True)
                return pb

            def bcast_sel(src64, n, tag):
                """row n of a (64,T) tile -> (128,TPAD) PSUM: DMA the row to a
                base-0 staging row, then K=1 broadcast matmul."""
                srow = sc2_pool.tile([1, T], F32, name=f"srow_{tag}", tag="srow", bufs=4)
                nc.sync.dma_start(srow[:], src64[n:n + 1, :])
                return bcast_row(srow, tag)

            def featsum(tiles, name, square=False, scale=1.0, bias=None):
                """(1,T) sbuf row = scale * sum_partitions(x or x^2) + bias."""
                out = sc2_pool.tile([1, T], F32, name=name, tag="fsrow")
                for (s, e) in CH:
                    ps = ps_pool.tile([1, 512], F32, name=f"{name}_ps", tag="pmm")
                    for j, xt in enumerate(tiles):
                        src = xt
                        if square:
                            sq = sc2_pool.tile([128, 512], F32, name=f"{name}_sq{j}",
                                               tag="sq")
                            nc.scalar.activation(sq[:, 0:e - s], xt[:, s:e], AT.Square)
                            src = sq
                            nc.tensor.matmul(ps[0:1, 0:e - s], _mm(ones_c[:]),
                                             _mm(sq[:, 0:e - s]),
                                             start=(j == 0), stop=(j == len(tiles) - 1))
                        else:
                            nc.tensor.matmul(ps[0:1, 0:e - s], _mm(ones_c[:]),
                                             _mm(src[:, s:e]),
                                             start=(j == 0), stop=(j == len(tiles) - 1))
                    nc.scalar.activation(out[0:1, s:e], ps[0:1, 0:e - s], AT.Identity,
                                         bias=(bias if bias is not None else 0.0),
                                         scale=scale)
                return out

            def rstd_row(ssum, name):
                sq = sc2_pool.tile([1, T], F32, name=name + "_sq", tag="fsrow")
                nc.scalar.activation(sq[:], ssum[:], AT.Sqrt)
                out = sc2_pool.tile([1, T], F32, name=name, tag="fsrow")
                nc.vector.reciprocal(out[:], sq[:])
                return out

            def load_slab(ext, col0, col1, kparts, name):
                """weight slab: K-chunk i in cols [128*i, 128*(i+1)) of the tile."""
                wt = wts_pool.tile([128, 128 * kparts], F32, name=name, tag="wip")
                for i in range(kparts):
                    nc.sync.dma_start(wt[:, 128 * i:128 * (i + 1)],
                                      ext[128 * i:128 * (i + 1), col0:col1])
                return wt

            def mm_into(dst_sbuf_ap_fn, slab, in_tiles, kparts, name,
                        act=AT.Identity, bias=0.0):
                """dst cols [s:e] = act(sum_i slab_i.T @ in_tiles[i][:,s:e] + bias)."""
                for (s, e) in CH:
                    pm = ps_pool.tile([128, 512], F32, name=f"{name}_pm", tag="pmm")
                    for i in range(kparts):
                        nc.tensor.matmul(pm[:, 0:e - s], _mm(slab[:, 128 * i:128 * (i + 1)]),
                                         _mm(in_tiles[i][:, s:e]),
                                         start=(i == 0), stop=(i == kparts - 1))
                    nc.scalar.activation(dst_sbuf_ap_fn(s, e), pm[:, 0:e - s], act, bias=bias)

            for l in range(2):
                rev = (l == 1)
                W = lambda nm: wext[(l, nm)]

                def final_ln(dst_ext):
                    m = featsum(R, f"lnm{l}", scale=1.0 / D_MODEL)
                    m_bc = bcast_row(m, "Bbc")
                    # centered squares summed without keeping all 8 xc tiles
                    sqsum = sc2_pool.tile([1, T], F32, name=f"lnv{l}", tag="fsrow")
                    for (s, e) in CH:
                        ps = ps_pool.tile([1, 512], F32, name=f"lnv{l}_ps", tag="pmm")
                        for j in range(NP):
                            xcj = sc2_pool.tile([128, 512], F32, name=f"lnxc{l}_{j}",
                                                tag="sq")
                            nc.vector.tensor_tensor(xcj[:, 0:e - s], R[j][:, s:e],
                                                    m_bc[:, s:e], OP.subtract)
                            nc.scalar.activation(xcj[:, 0:e - s], xcj[:, 0:e - s], AT.Square)
                            nc.tensor.matmul(ps[0:1, 0:e - s], _mm(ones_c[:]),
                                             _mm(xcj[:, 0:e - s]),
                                             start=(j == 0), stop=(j == NP - 1))
                        nc.scalar.activation(sqsum[0:1, s:e], ps[0:1, 0:e - s], AT.Identity,
                                             bias=eps_ln[0:1, 0:1], scale=1.0 / D_MODEL)
                    rs = rstd_row(sqsum, f"lnrs{l}")
                    rs_bc = bcast_row(rs, "Cbc")
                    for j in range(NP):
                        o = sc2_pool.tile([128, T], F32, name=f"lno{l}_{j}", tag="mo", bufs=1)
                        nc.vector.tensor_tensor(o[:], R[j][:], m_bc[:, 0:T], OP.subtract)
                        nc.vector.tensor_tensor(o[:], o[:], rs_bc[:, 0:T], OP.mult)
                        nc.vector.tensor_scalar(o[:], o[:], lnw[j][:, 0:1], lnb[j][:, 0:1],
                                                OP.mult, OP.add)
                        nc.sync.dma_start(dst_ext[128 * j:128 * (j + 1), :], o[:])

                if l == 1:
                    final_ln(y0_ext)

                # ---------- rmsnorm (norm weight folded into in_proj/fc1 on host)
                ss = featsum(R, f"rms{l}", square=True, scale=1.0 / D_MODEL, bias=eps_rms[0:1, 0:1])
                g = rstd_row(ss, f"rmsg{l}")
                g_bc = bcast_row(g, "Bbc")
                xn = [scr_pool.tile([128, T], F32, name=f"xn{l}_{j}", tag=f"xn{j}")
                      for j in range(NP)]
                for j in range(NP):
                    nc.vector.tensor_tensor(xn[j][:], R[j][:], g_bc[:, 0:T], OP.mult)

                # ---------- in_proj -> xm (conv-padded), z
                xm_pad = [act_pool.tile([128, T + 6], F32, name=f"xmp{l}_{i}", tag=f"xmp{i}")
                          for i in range(2)]
                z = [act_pool.tile([128, T], F32, name=f"z{l}_{i}", tag=f"z{i}")
                     for i in range(2)]
                for i in range(2):
                    nc.gpsimd.memset(xm_pad[i][:, 0:3], 0.0)
                    nc.gpsimd.memset(xm_pad[i][:, T + 3:T + 6], 0.0)
                for i in range(2):
                    slab = load_slab(W("w_xm"), 128 * i, 128 * (i + 1), NP, f"wxm{l}{i}")
                    mm_into(lambda s, e, i=i: xm_pad[i][:, 3 + s:3 + e], slab, xn, NP,
                            f"ipx{l}{i}")
                for i in range(2):
                    slab = load_slab(W("w_z"), 128 * i, 128 * (i + 1), NP, f"wz{l}{i}")
                    mm_into(lambda s, e, i=i: z[i][:, s:e], slab, xn, NP, f"ipz{l}{i}")

                # ---------- depthwise conv (anticausal for l=1; weights host-flipped) + silu
                convw = [wts_pool.tile([128, D_CONV], F32, name=f"convw{l}_{i}", tag=f"convw{i}")
                         for i in range(2)]
                convb = [wts_pool.tile([128, 1], F32, name=f"convb{l}_{i}", tag=f"convb{i}")
                         for i in range(2)]
                for i in range(2):
                    nc.sync.dma_start(convw[i][:], W("convw")[128 * i:128 * (i + 1), :])
                    nc.sync.dma_start(convb[i][:], W("convb")[128 * i:128 * (i + 1), :])
                u = [act_pool.tile([128, T], F32, name=f"u{l}_{i}", tag=f"u{i}")
                     for i in range(2)]
                base = 3 if rev else 0
                for i in range(2):
                    nc.vector.tensor_scalar(u[i][:], xm_pad[i][:, base:base + T],
                                            convw[i][:, 0:1], convb[i][:, 0:1],
                                            OP.mult, OP.add)
                    for k in range(1, D_CONV):
                        nc.vector.scalar_tensor_tensor(u[i][:],
                                                       xm_pad[i][:, base + k:base + k + T],
                                                       convw[i][:, k:k + 1], u[i][:],
                                                       OP.mult, OP.add)
                    nc.scalar.activation(u[i][:], u[i][:], AT.Silu)

                # ---------- x_proj partials -> AllReduce -> dtraw+B (128,T), C (64,T)
                xproj = [wts_pool.tile([128, 192], F32, name=f"xproj{l}_{i}", tag=f"xproj{i}")
                         for i in range(2)]
                for i in range(2):
                    nc.sync.dma_start(xproj[i][:], W("xproj")[128 * i:128 * (i + 1), :])
                bcp0 = act_pool.tile([128, T], F32, name=f"bcp0_{l}", tag="bcp0")
                bcp1 = act_pool.tile([64, T], F32, name=f"bcp1_{l}", tag="bcp1", bufs=2)
                for mi, (m0, m1, dst) in enumerate([(0, 64, None), (64, 128, None),
                                                    (128, 192, None)]):
                    for (s, e) in CH:
                        pm = ps_pool.tile([64, 512], F32, name=f"pbc{l}_{mi}", tag="pmm")
                        for i in range(2):
                            nc.tensor.matmul(pm[:, 0:e - s], _mm(xproj[i][:, m0:m1]),
                                             _mm(u[i][:, s:e]), start=(i == 0), stop=(i == 1))
                        if mi < 2:
                            nc.scalar.activation(bcp0[64 * mi:64 * (mi + 1), s:e],
                                                 pm[:, 0:e - s], AT.Identity)
                        else:
                            nc.scalar.activation(bcp1[0:64, s:e], pm[:, 0:e - s], AT.Identity)
                nc.sync.dma_start(bc_in[0:128, :], bcp0[:])
                nc.sync.dma_start(bc_in[128:192, :], bcp1[:])
                nc.gpsimd.collective_compute(
                    "AllReduce", OP.add, replica_groups=[list(range(NCORES))],
                    ins=[bc_in.opt()], outs=[bc_out.opt()])
                dtraw = act_pool.tile([64, T], F32, name=f"dtraw{l}", tag="bcp0")
                Bt = act_pool.tile([64, T], F32, name=f"Bt{l}", tag="bcp1", bufs=2)
                Ct = act_pool.tile([64, T], F32, name=f"Ct{l}", tag="bcp1", bufs=2)
                nc.sync.dma_start(dtraw[:], bc_out[0:64, :])
                nc.sync.dma_start(Bt[:], bc_out[64:128, :])
                nc.sync.dma_start(Ct[:], bc_out[128:192, :])

                # ---------- dt = softplus(dtraw @ dt_w + dt_b);  v = dt*u
                dtw = wts_pool.tile([DT_RANK, DLOC], F32, name=f"dtw{l}", tag="dtw")
                nc.sync.dma_start(dtw[:], W("dtw")[:])
                dtb = [wts_pool.tile([128, 1], F32, name=f"dtb{l}_{i}", tag=f"dtb{i}")
                       for i in range(2)]
                dvec = [wts_pool.tile([128, 1], F32, name=f"dv{l}_{i}", tag=f"dv{i}")
                        for i in range(2)]
                for i in range(2):
                    nc.sync.dma_start(dtb[i][:], W("dtb")[128 * i:128 * (i + 1), :])
                    nc.sync.dma_start(dvec[i][:], W("dvec")[128 * i:128 * (i + 1), :])
                dt = [act_pool.tile([128, T], F32, name=f"dt{l}_{i}", tag=f"dt{i}")
                      for i in range(2)]
                v = [act_pool.tile([128, T], F32, name=f"v{l}_{i}", tag=f"v{i}")
                     for i in range(2)]
                for i in range(2):
                    for (s, e) in CH:
                        pm = ps_pool.tile([128, 512], F32, name=f"pdt{l}_{i}", tag="pmm")
                        nc.tensor.matmul(pm[:, 0:e - s], _mm(dtw[:, 128 * i:128 * (i + 1)]),
                                         _mm(dtraw[:, s:e]), start=True, stop=True)
                        # softplus(x) = ln(1 + exp(x)); Softplus has no ACT table
                        nc.scalar.activation(dt[i][:, s:e], pm[:, 0:e - s], AT.Exp,
                                             bias=dtb[i][:, 0:1])
                        nc.scalar.activation(dt[i][:, s:e], dt[i][:, s:e], AT.Ln,
                                             bias=1.0)
                    nc.vector.tensor_tensor(v[i][:], dt[i][:], u[i][:], OP.mult)

                # ---------- selective scan over the 64 state dims
                Y = [act_pool.tile([128, T], F32, name=f"Y{l}_{i}", tag=f"Y{i}")
                     for i in range(2)]
                for i in range(2):
                    nc.gpsimd.memset(Y[i][:], 0.0)
                for n in range(D_STATE):
                    an = float(an_values[n])
                    B_bc = bcast_sel(Bt, n, "Bbc")
                    C_bc = bcast_sel(Ct, n, "Cbc")
                    for i in range(2):
                        a_t = sc2_pool.tile([128, T], F32, name="a_t", tag=f"a{i}", bufs=1)
                        b_t = sc2_pool.tile([128, T], F32, name="b_t", tag=f"b{i}", bufs=1)
                        h_t = sc2_pool.tile([128, T], F32, name="h_t", tag=f"h{i}", bufs=1)
                        nc.scalar.activation(a_t[:], dt[i][:], AT.Exp, scale=an)
                        nc.vector.tensor_tensor(b_t[:], v[i][:], B_bc[:, 0:T], OP.mult)
                        if rev:
                            nc.vector.tensor_tensor_scan(h_t[:, ::-1], a_t[:, ::-1],
                                                         b_t[:, ::-1], 0.0, OP.mult, OP.add)
                        else:
                            nc.vector.tensor_tensor_scan(h_t[:], a_t[:], b_t[:],
                                                         0.0, OP.mult, OP.add)
                        nc.vector.tensor_tensor(h_t[:], h_t[:], C_bc[:, 0:T], OP.mult)
                        nc.gpsimd.tensor_tensor(Y[i][:], Y[i][:], h_t[:], OP.add)

                if debug and l == 0:
                    for nm, ap in [("xn0", xn[0][:]), ("xm0", xm_pad[0][:, 3:T + 3]),
                                   ("u0", u[0][:]), ("dtraw", dtraw[:]), ("Bt", Bt[:]),
                                   ("Ct", Ct[:]), ("dt0", dt[0][:]), ("Y0", Y[0][:]),
                                   ("z0", z[0][:])]:
                        _mkdbg(nm, ap.shape)
                        nc.sync.dma_start(dbg_outs[nm][:], ap)

                # ---------- y = (Y + u*D) * silu(z); out_proj partials + AR + residual
                yg = [act_pool.tile([128, T], F32, name=f"yg{l}_{i}", tag=f"yg{i}")
                      for i in range(2)]
                for i in range(2):
                    nc.vector.scalar_tensor_tensor(Y[i][:], u[i][:], dvec[i][:, 0:1],
                                                   Y[i][:], OP.mult, OP.add)
                    nc.scalar.activation(z[i][:], z[i][:], AT.Silu)
                    nc.vector.tensor_tensor(yg[i][:], Y[i][:], z[i][:], OP.mult)

                def shard_matmul_to_R(in_tiles, w_ext_nm, bias_ext_nm, kparts, name):
                    for j in range(NP):
                        slab = wts_pool.tile([128, 128 * kparts], F32,
                                             name=f"{name}_w{j}", tag="wip")
                        for i in range(kparts):
                            nc.sync.dma_start(slab[:, 128 * i:128 * (i + 1)],
                                              W(w_ext_nm)[128 * i:128 * (i + 1),
                                                          128 * j:128 * (j + 1)])
                        mo = sc2_pool.tile([128, T], F32, name=f"{name}_mo{j}", tag="mo", bufs=1)
                        if bias_ext_nm is None:
                            mm_into(lambda s, e, mo=mo: mo[:, s:e], slab, in_tiles,
                                    kparts, f"{name}{j}")
                        else:
                            bt = wts_pool.tile([128, 1], F32, name=f"{name}_b{j}", tag="bmo")
                            nc.sync.dma_start(bt[:], W(bias_ext_nm)[128 * j:128 * (j + 1), :])
                            for (s, e) in CH:
                                pm = ps_pool.tile([128, 512], F32, name=f"{name}_pm{j}",
                                                  tag="pmm")
                                for i in range(kparts):
                                    nc.tensor.matmul(pm[:, 0:e - s],
                                                     _mm(slab[:, 128 * i:128 * (i + 1)]),
                                                     _mm(in_tiles[i][:, s:e]),
                                                     start=(i == 0), stop=(i == kparts - 1))
                                nc.scalar.activation(mo[:, s:e], pm[:, 0:e - s], AT.Identity,
                                                     bias=bt[:, 0:1])
                        nc.sync.dma_start(ar_in[128 * j:128 * (j + 1), :], mo[:])
                    nc.gpsimd.collective_compute(
                        "AllReduce", OP.add, replica_groups=[list(range(NCORES))],
                        ins=[ar_in.opt()], outs=[ar_out.opt()])
                    for j in range(NP):
                        hm = sc2_pool.tile([128, T], F32, name=f"{name}_hm{j}", tag="mo", bufs=1)
                        nc.sync.dma_start(hm[:], ar_out[128 * j:128 * (j + 1), :])
                        nc.vector.tensor_tensor(R[j][:], R[j][:], hm[:], OP.add)

                if debug and l == 0:
                    _mkdbg("yg0", yg[0].shape)
                    nc.sync.dma_start(dbg_outs["yg0"][:], yg[0][:])
                shard_matmul_to_R(yg, "wout", None, DLOC // 128, f"op{l}")
                if debug and l == 0:
                    _mkdbg("R0_op", R[0].shape)
                    nc.sync.dma_start(dbg_outs["R0_op"][:], R[0][:])

                # ---------- MLP
                ss2 = featsum(R, f"rms2{l}", square=True, scale=1.0 / D_MODEL, bias=eps_rms[0:1, 0:1])
                g2 = rstd_row(ss2, f"rmsg2{l}")
                g2_bc = bcast_row(g2, "Bbc")
                xn2 = [scr_pool.tile([128, T], F32, name=f"xn2{l}_{j}", tag=f"xn{j}")
                       for j in range(NP)]
                for j in range(NP):
                    nc.vector.tensor_tensor(xn2[j][:], R[j][:], g2_bc[:, 0:T], OP.mult)

                gated = [act_pool.tile([128, T], F32, name=f"gt{l}_{i}", tag=f"yg{i}")
                         for i in range(2)]
                for i in range(2):
                    f1by = wts_pool.tile([128, 1], F32, name=f"f1by{l}{i}", tag="f1by")
                    f1bg = wts_pool.tile([128, 1], F32, name=f"f1bg{l}{i}", tag="f1bg")
                    nc.sync.dma_start(f1by[:], W("fc1by")[128 * i:128 * (i + 1), :])
                    nc.sync.dma_start(f1bg[:], W("fc1bg")[128 * i:128 * (i + 1), :])
                    slab_y = load_slab(W("fc1y"), 128 * i, 128 * (i + 1), NP, f"fy{l}{i}")
                    yh = act_pool.tile([128, T], F32, name=f"yh{l}_{i}", tag="xmp0", bufs=1)
                    for (s, e) in CH:
                        pm = ps_pool.tile([128, 512], F32, name=f"py{l}{i}", tag="pmm")
                        for j in range(NP):
                            nc.tensor.matmul(pm[:, 0:e - s],
                                             _mm(slab_y[:, 128 * j:128 * (j + 1)]),
                                             _mm(xn2[j][:, s:e]),
                                             start=(j == 0), stop=(j == NP - 1))
                        nc.scalar.activation(yh[:, s:e], pm[:, 0:e - s], AT.Identity,
                                             bias=f1by[:, 0:1])
                    slab_g = load_slab(W("fc1g"), 128 * i, 128 * (i + 1), NP, f"fg{l}{i}")
                    gh = act_pool.tile([128, T], F32, name=f"gh{l}_{i}", tag="xmp1", bufs=1)
                    for (s, e) in CH:
                        pm = ps_pool.tile([128, 512], F32, name=f"pg{l}{i}", tag="pmm")
                        for j in range(NP):
                            nc.tensor.matmul(pm[:, 0:e - s],
                                             _mm(slab_g[:, 128 * j:128 * (j + 1)]),
                                             _mm(xn2[j][:, s:e]),
                                             start=(j == 0), stop=(j == NP - 1))
                        nc.scalar.activation(gh[:, s:e], pm[:, 0:e - s], AT.Silu,
                                             bias=f1bg[:, 0:1])
                    nc.vector.tensor_tensor(gated[i][:], yh[:], gh[:], OP.mult)

                shard_matmul_to_R(gated, "fc2", "fc2b", HLOC // 128, f"mlp{l}")

                if l == 1:
                    final_ln(y1_ext)

    return nc


MAXW = 1


def _split_waits(nc):
    """This walrus build rejects >1 sem wait on CTRL instructions; hoist
    extras onto preceding nops on the same engine."""
    for bb in nc.main_func.blocks:
        insts = list(bb.instructions)
        out = []
        for inst in insts:
            si = inst.sync_info
            w = list(si.on_wait) if (si and si.on_wait) else []
            if len(w) > MAXW:
                extra, keep = w[:-MAXW], w[-MAXW:]
                for i0 in range(0, len(extra), MAXW):
                    nop = mybir.InstNoOp(name=f"{inst.name}-wsplit-{i0}", ins=[], outs=[])
                    nop.engine = inst.engine
                    nop.sync_info = mybir.SyncInfo(on_wait=extra[i0:i0 + MAXW], on_update=[])
                    out.append(nop)
                si.on_wait = keep
            out.append(inst)
        bb.instructions[:] = out


def make_inputs(hidden_states, params, T=None):
    hs = np.asarray(hidden_states, np.float32)
    assert hs.shape[0] == 1
    sos = np.asarray(params["sos"], np.float32)[None, :]
    eos = np.asarray(params["eos"], np.float32)[None, :]
    x = np.concatenate([sos, hs[0], eos], 0)
    if T is not None:
        x = x[:T]
    T = x.shape[0]

    layers = params["layers"]
    A = -np.exp(np.asarray(layers[0]["A_log"], np.float32))
    assert np.allclose(A, A[0:1, :]), "kernel assumes d-independent A"
    an_values = A[0].astype(np.float32)

    base = {
        "xT": np.ascontiguousarray(x.T),
        "ones_r": np.ones((128, 128), np.float32),
        "ones_c": np.ones((128, 1), np.float32),
                "lnw": np.asarray(params["ln_f_w"], np.float32)[:, None],
        "lnb": np.asarray(params["ln_f_b"], np.float32)[:, None],
    }
    in_maps = []
    for c in range(NCORES):
        m = dict(base)
        sl = slice(c * DLOC, (c + 1) * DLOC)
        hl = slice(c * HLOC, (c + 1) * HLOC)
        for l, p in enumerate(layers):
            n1 = np.asarray(p["norm1_w"], np.float32)
            n2 = np.asarray(p["norm2_w"], np.float32)
            in_proj = np.asarray(p["in_proj"], np.float32) * n1[:, None]
            convw = np.asarray(p["conv_w"], np.float32)[sl]
            if l == 1:
                convw = convw[:, ::-1]
            Al = -np.exp(np.asarray(p["A_log"], np.float32))
            assert np.allclose(Al, A[0:1, :])
            fc1 = np.asarray(p["fc1_w"], np.float32) * n2[:, None]
            fc1b = np.asarray(p["fc1_b"], np.float32)
            m.update({
                f"L{l}_w_xm": np.ascontiguousarray(in_proj[:, :D_INNER][:, sl]),
                f"L{l}_w_z": np.ascontiguousarray(in_proj[:, D_INNER:][:, sl]),
                f"L{l}_convw": np.ascontiguousarray(convw),
                f"L{l}_convb": np.asarray(p["conv_b"], np.float32)[sl][:, None],
                f"L{l}_xproj": np.ascontiguousarray(np.asarray(p["x_proj"], np.float32)[sl]),
                f"L{l}_dtw": np.ascontiguousarray(np.asarray(p["dt_w"], np.float32)[:, sl]),
                f"L{l}_dtb": np.asarray(p["dt_b"], np.float32)[sl][:, None],
                f"L{l}_dvec": np.asarray(p["D"], np.float32)[sl][:, None],
                f"L{l}_wout": np.ascontiguousarray(np.asarray(p["out_proj"], np.float32)[sl]),
                f"L{l}_fc1y": np.ascontiguousarray(fc1[:, :HIDDEN][:, hl]),
                f"L{l}_fc1g": np.ascontiguousarray(fc1[:, HIDDEN:][:, hl]),
                f"L{l}_fc1by": fc1b[:HIDDEN][hl][:, None],
                f"L{l}_fc1bg": fc1b[HIDDEN:][hl][:, None],
                f"L{l}_fc2": np.ascontiguousarray(np.asarray(p["fc2_w"], np.float32)[hl]),
                f"L{l}_fc2b": (np.asarray(p["fc2_b"], np.float32) / NCORES)[:, None],
            })
        in_maps.append(m)
    return in_maps, T, an_values


_CACHE = {}


def kernel(hidden_states, params):
    in_maps, T, an_values = make_inputs(hidden_states, params)
    key = ("k", T, MM_MODE)
    if key not in _CACHE:
        nc = bass.Bass()
        build_kernel(nc, T, an_values)
        _split_waits(nc)
        _CACHE[key] = nc
    nc = _CACHE[key]
    res = run_bass_kernel_spmd(nc, in_maps, list(range(NCORES)))
    y0 = res.results[0]["y0T"].T[None]
    y1 = np.ascontiguousarray(res.results[0]["y1T"].T[None][:, ::-1])
    return y0, y1
